# revision 28
# baseline (speedup 1.0000x reference)
"""Block-circulant linear layer on TRN2 via two-level circulant CRT split.

y[n, j*B+k] = sum_{i,b} c[j,i,(k-b) mod B] * x[n, i*B+b] + bias[j*B+k]

Level 1: x^256-1 = (x^128-1)(x^128+1) -> cyclic-128 system U (on u) and
negacyclic-128 system V (on v). Level 2 splits U again:
x^128-1 = (x^64-1)(x^64+1) -> UU (cyclic-64, on uu), UV (negacyclic-64,
on uv). Matmul FLOPs drop to 3/8 of the dense 4096x4096 form:
  yv  = v  @ V/2  + beta_v    (2048x2048)
  yuu = uu @ UU/4 + beta_uu   (1024x1024)
  yuv = uv @ UV/4 + beta_uv   (1024x1024)
  yu_lo = yuu + yuv, yu_hi = yuu - yuv          (stage A)
  y_lo = yu + yv, y_hi = yu - yv                (stage B)

Sharding: data-parallel over the 8192 tokens (1024/core); weights
replicated. fp32r (e8m11) matmul datapath; bias folded in via K=1
ones-row matmuls; input butterflies/transpose and output reassembly are
host-side data marshalling.
"""

import numpy as np

import concourse.bass as bass
import concourse.mybir as mybir
import concourse.tile as tile
from concourse import bacc
from concourse.bass_utils import run_bass_kernel_spmd

B = 256
H = B // 2               # 128
Q = B // 4               # 64
IN_BLOCKS = 16
OUT_BLOCKS = 16
BATCH, SEQ = 4, 2048
IN_F = IN_BLOCKS * B     # 4096
OUT_F = OUT_BLOCKS * B   # 4096
HF = IN_BLOCKS * H       # 2048 (V system width)
QF = IN_BLOCKS * Q       # 1024 (UU/UV system width)
N_CORES = 8
NTOK = BATCH * SEQ       # 8192
TOK = NTOK // N_CORES    # 1024 tokens per core

KTV = HF // 128          # 16 contraction tiles, V system
KTQ = QF // 128          # 8 contraction tiles, UU/UV systems
MT = TOK // 128          # 8 token tiles
NW = 512                 # moving free dim per matmul (one psum bank)
NTV = HF // NW           # 4 column chunks, V system
NTQ = QF // NW           # 2 column chunks, UU/UV systems
JB = NW // H             # 4 j-blocks per V/output chunk

_NC_CACHE = {}


def _build_nc():
    f32 = mybir.dt.float32
    f32r = mybir.dt.float32r

    nc = bacc.Bacc("TRN2", target_bir_lowering=False, debug=False)
    vT = nc.dram_tensor("vT", [HF, TOK], f32r, kind="ExternalInput")
    uuT = nc.dram_tensor("uuT", [QF, TOK], f32r, kind="ExternalInput")
    uvT = nc.dram_tensor("uvT", [QF, TOK], f32r, kind="ExternalInput")
    wV = nc.dram_tensor("wV", [NTV, KTV, 128, NW], f32r, kind="ExternalInput")
    wUU = nc.dram_tensor("wUU", [NTQ, KTQ, 128, NW], f32r, kind="ExternalInput")
    wUV = nc.dram_tensor("wUV", [NTQ, KTQ, 128, NW], f32r, kind="ExternalInput")
    # rows: 0 = beta_v (2048), 32 = beta_uu (1024 + pad), 64 = beta_uv
    # (matmul operands must start at partition 0, 32, or 64)
    betaAll = nc.dram_tensor("betaAll", [65, HF], f32r, kind="ExternalInput")
    ones = nc.dram_tensor("ones", [65, 128], f32r, kind="ExternalInput")
    # y stored as raw stage-B tiles (n, m, lo/hi, 128, NW); host reassembles
    y = nc.dram_tensor(
        "y", [NTV, MT, 2, 128, NW], f32, kind="ExternalOutput"
    )

    with tile.TileContext(nc) as tc:
        with (
            tc.tile_pool(name="inpool", bufs=1) as inpool,
            tc.tile_pool(name="cpool", bufs=1) as cpool,
            tc.tile_pool(name="wpool", bufs=10) as wpool,
            tc.tile_pool(name="yupool", bufs=8) as yupool,
            tc.tile_pool(name="ycpool", bufs=2) as ycpool,
            tc.tile_pool(name="ypool", bufs=2) as ypool,
            tc.tile_pool(name="psum", bufs=8, space="PSUM") as psum_pool,
        ):
            # Small consts first on the load queue.
            ones_sb = cpool.tile([65, 128], f32r, tag="ones")
            nc.sync.dma_start(out=ones_sb[:], in_=ones[:, :])
            beta_sb = cpool.tile([65, HF], f32r, tag="beta")
            for r in (0, 32, 64):
                nc.sync.dma_start(
                    out=beta_sb[r : r + 1, :], in_=betaAll[r : r + 1, :]
                )

            # Input k-tiles are loaded lazily, interleaved with the W
            # stream in exact consumption order, all on the fast
            # sync-issued HWDGE queue (side-engine queues run ~4x slower).
            in_tiles = {}

            def get_input(which, dram, i):
                key = (which, i)
                if key not in in_tiles:
                    t = inpool.tile(
                        [128, TOK], f32r, tag=f"{which}{i}", name=f"{which}{i}"
                    )
                    nc.sync.dma_start(
                        out=t[:], in_=dram[i * 128 : (i + 1) * 128, :]
                    )
                    in_tiles[key] = t
                return in_tiles[key]

            def system_phase(which, dram, ktiles, wdram, nn, beta_row):
                """One accumulation phase: psum[m] = sum_k lhsT_k.T @ W + beta."""
                ps = [
                    psum_pool.tile(
                        [128, NW], f32, tag="ps", name=f"ps_{which}_{nn}_{m}"
                    )
                    for m in range(MT)
                ]
                for k in range(ktiles):
                    lhs = get_input(which, dram, k)
                    wt = wpool.tile(
                        [128, NW], f32r, tag="w", name=f"w_{which}_{nn}_{k}"
                    )
                    nc.sync.dma_start(out=wt[:], in_=wdram[nn, k, :, :])
                    last = k == ktiles - 1
                    for m in range(MT):
                        nc.tensor.matmul(
                            ps[m][:],
                            lhs[:, m * 128 : (m + 1) * 128],
                            wt[:],
                            start=(k == 0),
                            stop=False,
                        )
                        if last:
                            # bias fold; closing the group here lets the
                            # m-th recombine start before the phase ends
                            nc.tensor.matmul(
                                ps[m][:],
                                ones_sb[beta_row : beta_row + 1, :],
                                beta_sb[
                                    beta_row : beta_row + 1,
                                    nn * NW : (nn + 1) * NW,
                                ],
                                start=False,
                                stop=True,
                            )
                return ps

            for nn in range(NTQ):
                psUU = system_phase("uu", uuT, KTQ, wUU, nn, 32)
                yc = []
                for m in range(MT):
                    t = ycpool.tile([128, NW], f32, tag="yc", name=f"yc_{nn}_{m}")
                    nc.vector.tensor_copy(t[:], psUU[m][:])
                    yc.append(t)
                psUV = system_phase("uv", uvT, KTQ, wUV, nn, 64)
                # stage A into a combined (j8, kk128) tile so stage B is
                # two full-width ops
                yu = []
                for m in range(MT):
                    t = yupool.tile(
                        [128, 2 * NW], f32, tag="yu", name=f"yu_{nn}_{m}"
                    )
                    yu3 = t[:].rearrange("p (j k) -> p j k", k=H)
                    yc3 = yc[m][:].rearrange("p (j k) -> p j k", k=Q)
                    puv3 = psUV[m][:].rearrange("p (j k) -> p j k", k=Q)
                    nc.vector.tensor_add(yu3[:, :, 0:Q], yc3, puv3)
                    nc.vector.tensor_sub(yu3[:, :, Q:H], yc3, puv3)
                    yu.append(t)
                for h in range(2):
                    n = 2 * nn + h
                    psV = system_phase("v", vT, KTV, wV, n, 0)
                    for m in range(MT):
                        tlo = ypool.tile(
                            [128, NW], f32, tag="tlo", name=f"tlo_{n}_{m}"
                        )
                        thi = ypool.tile(
                            [128, NW], f32, tag="thi", name=f"thi_{n}_{m}"
                        )
                        yslice = yu[m][:, h * NW : (h + 1) * NW]
                        nc.vector.tensor_add(tlo[:], yslice, psV[m][:])
                        nc.vector.tensor_sub(thi[:], yslice, psV[m][:])
                        eng = nc.gpsimd if m % 2 == 0 else nc.scalar
                        eng.dma_start(out=y[n, m, 0, :, :], in_=tlo[:])
                        eng.dma_start(out=y[n, m, 1, :, :], in_=thi[:])
    nc.finalize()
    return nc


def _get_nc():
    if "nc" not in _NC_CACHE:
        _NC_CACHE["nc"] = _build_nc()
    return _NC_CACHE["nc"]


def _round_fp32r(a: np.ndarray) -> np.ndarray:
    """Round fp32 to fp32r (e8m11: low 12 mantissa bits zero), RNE."""
    u = np.ascontiguousarray(a, dtype=np.float32).view(np.uint32)
    r = (u + (0x7FF + ((u >> 12) & 1))) & np.uint32(0xFFFFF000)
    return r.view(np.float32)


def _cyc(cm, n):
    k = np.arange(n)
    b = np.arange(n)
    return cm[:, :, (k[None] - b[:, None]) % n]


def _neg(cm, n):
    k = np.arange(n)
    b = np.arange(n)
    s = np.where(k[None] >= b[:, None], 1.0, -1.0).astype(np.float32)
    return cm[:, :, (k[None] - b[:, None]) % n] * s[None, None]


def _flat(blk, n):
    # (j, i, bb, kk) -> (I*n, J*n)
    return blk.transpose(1, 2, 0, 3).reshape(IN_BLOCKS * n, OUT_BLOCKS * n)


def _tiled(w, nt, kt):
    # (K, N) -> (nt, kt, 128, NW): each [128, NW] tile contiguous
    return np.ascontiguousarray(
        w.reshape(kt, 128, nt, NW).transpose(2, 0, 1, 3)
    )


def _build_weights(c: np.ndarray, bias: np.ndarray):
    cu = c[:, :, :H] + c[:, :, H:]
    cv = c[:, :, :H] - c[:, :, H:]
    cuu = cu[:, :, :Q] + cu[:, :, Q:]
    cuv = cu[:, :, :Q] - cu[:, :, Q:]

    V = _flat(_neg(cv, H), H) * 0.5
    UU = _flat(_cyc(cuu, Q), Q) * 0.25
    UV = _flat(_neg(cuv, Q), Q) * 0.25

    bias_b = bias.reshape(OUT_BLOCKS, B)
    bu = 0.5 * (bias_b[:, :H] + bias_b[:, H:])           # (J, H)
    bv = 0.5 * (bias_b[:, :H] - bias_b[:, H:]).reshape(OUT_BLOCKS * H)
    buu = 0.5 * (bu[:, :Q] + bu[:, Q:]).reshape(OUT_BLOCKS * Q)
    buv = 0.5 * (bu[:, :Q] - bu[:, Q:]).reshape(OUT_BLOCKS * Q)
    betaAll = np.zeros((65, HF), np.float32)
    betaAll[0] = bv
    betaAll[32, :QF] = buu
    betaAll[64, :QF] = buv

    return (
        _round_fp32r(_tiled(V, NTV, KTV)),
        _round_fp32r(_tiled(UU, NTQ, KTQ)),
        _round_fp32r(_tiled(UV, NTQ, KTQ)),
        _round_fp32r(betaAll),
    )


def kernel(x, c, bias, _spmd_kwargs=None):
    x = np.asarray(x, dtype=np.float32)
    c = np.asarray(c, dtype=np.float32)
    bias = np.asarray(bias, dtype=np.float32)

    wv, wuu, wuv, betas = _build_weights(c, bias)
    ones = np.ones((65, 128), dtype=np.float32)

    xb = x.reshape(NTOK, IN_BLOCKS, B)
    u = xb[:, :, :H] + xb[:, :, H:]                      # (NTOK, I, H)
    v_all = (xb[:, :, :H] - xb[:, :, H:]).reshape(NTOK, HF)
    uu_all = (u[:, :, :Q] + u[:, :, Q:]).reshape(NTOK, QF)
    uv_all = (u[:, :, :Q] - u[:, :, Q:]).reshape(NTOK, QF)

    in_maps = []
    for cid in range(N_CORES):
        sl = slice(cid * TOK, (cid + 1) * TOK)
        in_maps.append(
            {
                "vT": _round_fp32r(v_all[sl].T),         # (HF, TOK)
                "uuT": _round_fp32r(uu_all[sl].T),       # (QF, TOK)
                "uvT": _round_fp32r(uv_all[sl].T),
                "wV": wv,
                "wUU": wuu,
                "wUV": wuv,
                "betaAll": betas,
                "ones": ones,
            }
        )

    nc = _get_nc()
    kw = dict(_spmd_kwargs or {})
    one_core = kw.pop("_one_core", False)
    if one_core:
        res = run_bass_kernel_spmd(nc, in_maps[:1], core_ids=[0], **kw)
        return None, res

    res = run_bass_kernel_spmd(
        nc, in_maps, core_ids=list(range(N_CORES)), **kw
    )

    def reassemble(a):
        # (NTV, MT, 2, 128, NW) -> (TOK, OUT_F)
        a = a.reshape(NTV, MT, 2, 128, JB, H)
        return a.transpose(1, 3, 0, 4, 2, 5).reshape(TOK, OUT_F)

    y = np.concatenate([reassemble(r["y"]) for r in res.results], axis=0)
    out = y.reshape(BATCH, SEQ, OUT_F)
    if _spmd_kwargs:
        return out, res
    return out


# revision 29
# speedup vs baseline: 1.0154x; 1.0154x over previous
"""Block-circulant linear layer on TRN2 via two-level circulant CRT split.

y[n, j*B+k] = sum_{i,b} c[j,i,(k-b) mod B] * x[n, i*B+b] + bias[j*B+k]

Level 1: x^256-1 = (x^128-1)(x^128+1) -> cyclic-128 system U (on u) and
negacyclic-128 system V (on v). Level 2 splits U again:
x^128-1 = (x^64-1)(x^64+1) -> UU (cyclic-64, on uu), UV (negacyclic-64,
on uv). Matmul FLOPs drop to 3/8 of the dense 4096x4096 form:
  yv  = v  @ V/2  + beta_v    (2048x2048)
  yuu = uu @ UU/4 + beta_uu   (1024x1024)
  yuv = uv @ UV/4 + beta_uv   (1024x1024)
  yu_lo = yuu + yuv, yu_hi = yuu - yuv          (stage A)
  y_lo = yu + yv, y_hi = yu - yv                (stage B)

Sharding: data-parallel over the 8192 tokens (1024/core); weights
replicated. fp32r (e8m11) matmul datapath; bias folded in via K=1
ones-row matmuls; input butterflies/transpose and output reassembly are
host-side data marshalling.
"""

import numpy as np

import concourse.bass as bass
import concourse.mybir as mybir
import concourse.tile as tile
from concourse import bacc
from concourse.bass_utils import run_bass_kernel_spmd

B = 256
H = B // 2               # 128
Q = B // 4               # 64
IN_BLOCKS = 16
OUT_BLOCKS = 16
BATCH, SEQ = 4, 2048
IN_F = IN_BLOCKS * B     # 4096
OUT_F = OUT_BLOCKS * B   # 4096
HF = IN_BLOCKS * H       # 2048 (V system width)
QF = IN_BLOCKS * Q       # 1024 (UU/UV system width)
N_CORES = 8
NTOK = BATCH * SEQ       # 8192
TOK = NTOK // N_CORES    # 1024 tokens per core

KTV = HF // 128          # 16 contraction tiles, V system
KTQ = QF // 128          # 8 contraction tiles, UU/UV systems
MT = TOK // 128          # 8 token tiles
NW = 512                 # moving free dim per matmul (one psum bank)
NTV = HF // NW           # 4 column chunks, V system
NTQ = QF // NW           # 2 column chunks, UU/UV systems
JB = NW // H             # 4 j-blocks per V/output chunk

_NC_CACHE = {}


def _build_nc():
    f32 = mybir.dt.float32
    f32r = mybir.dt.float32r

    nc = bacc.Bacc("TRN2", target_bir_lowering=False, debug=False)
    vT = nc.dram_tensor("vT", [HF, TOK], f32r, kind="ExternalInput")
    uuT = nc.dram_tensor("uuT", [QF, TOK], f32r, kind="ExternalInput")
    uvT = nc.dram_tensor("uvT", [QF, TOK], f32r, kind="ExternalInput")
    wV = nc.dram_tensor("wV", [NTV, KTV, 128, NW], f32r, kind="ExternalInput")
    wUU = nc.dram_tensor("wUU", [NTQ, KTQ, 128, NW], f32r, kind="ExternalInput")
    wUV = nc.dram_tensor("wUV", [NTQ, KTQ, 128, NW], f32r, kind="ExternalInput")
    # rows: 0 = beta_v (2048), 32 = beta_uu (1024 + pad), 64 = beta_uv
    # (matmul operands must start at partition 0, 32, or 64)
    betaAll = nc.dram_tensor("betaAll", [65, HF], f32r, kind="ExternalInput")
    ones = nc.dram_tensor("ones", [65, 128], f32r, kind="ExternalInput")
    # y stored as raw stage-B tiles (n, m, lo/hi, 128, NW); host reassembles
    y = nc.dram_tensor(
        "y", [NTV, MT, 2, 128, NW], f32, kind="ExternalOutput"
    )

    with tile.TileContext(nc) as tc:
        with (
            tc.tile_pool(name="inpool", bufs=1) as inpool,
            tc.tile_pool(name="cpool", bufs=1) as cpool,
            tc.tile_pool(name="wpool", bufs=10) as wpool,
            tc.tile_pool(name="yupool", bufs=8) as yupool,
            tc.tile_pool(name="ycpool", bufs=3) as ycpool,
            tc.tile_pool(name="ypool", bufs=2) as ypool,
            tc.tile_pool(name="psum", bufs=8, space="PSUM") as psum_pool,
        ):
            # Small consts first on the load queue.
            ones_sb = cpool.tile([65, 128], f32r, tag="ones")
            nc.sync.dma_start(out=ones_sb[:], in_=ones[:, :])
            beta_sb = cpool.tile([65, HF], f32r, tag="beta")
            for r in (0, 32, 64):
                nc.sync.dma_start(
                    out=beta_sb[r : r + 1, :], in_=betaAll[r : r + 1, :]
                )

            # Input k-tiles are loaded lazily, interleaved with the W
            # stream in exact consumption order, all on the fast
            # sync-issued HWDGE queue (side-engine queues run ~4x slower).
            in_tiles = {}

            def get_input(which, dram, i):
                key = (which, i)
                if key not in in_tiles:
                    t = inpool.tile(
                        [128, TOK], f32r, tag=f"{which}{i}", name=f"{which}{i}"
                    )
                    nc.sync.dma_start(
                        out=t[:], in_=dram[i * 128 : (i + 1) * 128, :]
                    )
                    in_tiles[key] = t
                return in_tiles[key]

            def system_phase(which, dram, ktiles, wdram, nn, beta_row):
                """One accumulation phase: psum[m] = sum_k lhsT_k.T @ W + beta."""
                ps = [
                    psum_pool.tile(
                        [128, NW], f32, tag="ps", name=f"ps_{which}_{nn}_{m}"
                    )
                    for m in range(MT)
                ]
                for k in range(ktiles):
                    lhs = get_input(which, dram, k)
                    wt = wpool.tile(
                        [128, NW], f32r, tag="w", name=f"w_{which}_{nn}_{k}"
                    )
                    nc.sync.dma_start(out=wt[:], in_=wdram[nn, k, :, :])
                    last = k == ktiles - 1
                    for m in range(MT):
                        nc.tensor.matmul(
                            ps[m][:],
                            lhs[:, m * 128 : (m + 1) * 128],
                            wt[:],
                            start=(k == 0),
                            stop=False,
                        )
                        if last:
                            # bias fold; closing the group here lets the
                            # m-th recombine start before the phase ends
                            nc.tensor.matmul(
                                ps[m][:],
                                ones_sb[beta_row : beta_row + 1, :],
                                beta_sb[
                                    beta_row : beta_row + 1,
                                    nn * NW : (nn + 1) * NW,
                                ],
                                start=False,
                                stop=True,
                            )
                return ps

            for nn in range(NTQ):
                psUU = system_phase("uu", uuT, KTQ, wUU, nn, 32)
                yc = []
                for m in range(MT):
                    t = ycpool.tile([128, NW], f32, tag="yc", name=f"yc_{nn}_{m}")
                    nc.vector.tensor_copy(t[:], psUU[m][:])
                    yc.append(t)
                psUV = system_phase("uv", uvT, KTQ, wUV, nn, 64)
                # stage A into a combined (j8, kk128) tile so stage B is
                # two full-width ops
                yu = []
                for m in range(MT):
                    t = yupool.tile(
                        [128, 2 * NW], f32, tag="yu", name=f"yu_{nn}_{m}"
                    )
                    yu3 = t[:].rearrange("p (j k) -> p j k", k=H)
                    yc3 = yc[m][:].rearrange("p (j k) -> p j k", k=Q)
                    puv3 = psUV[m][:].rearrange("p (j k) -> p j k", k=Q)
                    nc.vector.tensor_add(yu3[:, :, 0:Q], yc3, puv3)
                    nc.vector.tensor_sub(yu3[:, :, Q:H], yc3, puv3)
                    yu.append(t)
                for h in range(2):
                    n = 2 * nn + h
                    psV = system_phase("v", vT, KTV, wV, n, 0)
                    for m in range(MT):
                        tlo = ypool.tile(
                            [128, NW], f32, tag="tlo", name=f"tlo_{n}_{m}"
                        )
                        thi = ypool.tile(
                            [128, NW], f32, tag="thi", name=f"thi_{n}_{m}"
                        )
                        yslice = yu[m][:, h * NW : (h + 1) * NW]
                        nc.vector.tensor_add(tlo[:], yslice, psV[m][:])
                        nc.vector.tensor_sub(thi[:], yslice, psV[m][:])
                        eng = nc.gpsimd if m % 2 == 0 else nc.scalar
                        eng.dma_start(out=y[n, m, 0, :, :], in_=tlo[:])
                        eng.dma_start(out=y[n, m, 1, :, :], in_=thi[:])
    nc.finalize()
    return nc


def _get_nc():
    if "nc" not in _NC_CACHE:
        _NC_CACHE["nc"] = _build_nc()
    return _NC_CACHE["nc"]


def _round_fp32r(a: np.ndarray) -> np.ndarray:
    """Round fp32 to fp32r (e8m11: low 12 mantissa bits zero), RNE."""
    u = np.ascontiguousarray(a, dtype=np.float32).view(np.uint32)
    r = (u + (0x7FF + ((u >> 12) & 1))) & np.uint32(0xFFFFF000)
    return r.view(np.float32)


def _cyc(cm, n):
    k = np.arange(n)
    b = np.arange(n)
    return cm[:, :, (k[None] - b[:, None]) % n]


def _neg(cm, n):
    k = np.arange(n)
    b = np.arange(n)
    s = np.where(k[None] >= b[:, None], 1.0, -1.0).astype(np.float32)
    return cm[:, :, (k[None] - b[:, None]) % n] * s[None, None]


def _flat(blk, n):
    # (j, i, bb, kk) -> (I*n, J*n)
    return blk.transpose(1, 2, 0, 3).reshape(IN_BLOCKS * n, OUT_BLOCKS * n)


def _tiled(w, nt, kt):
    # (K, N) -> (nt, kt, 128, NW): each [128, NW] tile contiguous
    return np.ascontiguousarray(
        w.reshape(kt, 128, nt, NW).transpose(2, 0, 1, 3)
    )


def _build_weights(c: np.ndarray, bias: np.ndarray):
    cu = c[:, :, :H] + c[:, :, H:]
    cv = c[:, :, :H] - c[:, :, H:]
    cuu = cu[:, :, :Q] + cu[:, :, Q:]
    cuv = cu[:, :, :Q] - cu[:, :, Q:]

    V = _flat(_neg(cv, H), H) * 0.5
    UU = _flat(_cyc(cuu, Q), Q) * 0.25
    UV = _flat(_neg(cuv, Q), Q) * 0.25

    bias_b = bias.reshape(OUT_BLOCKS, B)
    bu = 0.5 * (bias_b[:, :H] + bias_b[:, H:])           # (J, H)
    bv = 0.5 * (bias_b[:, :H] - bias_b[:, H:]).reshape(OUT_BLOCKS * H)
    buu = 0.5 * (bu[:, :Q] + bu[:, Q:]).reshape(OUT_BLOCKS * Q)
    buv = 0.5 * (bu[:, :Q] - bu[:, Q:]).reshape(OUT_BLOCKS * Q)
    betaAll = np.zeros((65, HF), np.float32)
    betaAll[0] = bv
    betaAll[32, :QF] = buu
    betaAll[64, :QF] = buv

    return (
        _round_fp32r(_tiled(V, NTV, KTV)),
        _round_fp32r(_tiled(UU, NTQ, KTQ)),
        _round_fp32r(_tiled(UV, NTQ, KTQ)),
        _round_fp32r(betaAll),
    )


def kernel(x, c, bias, _spmd_kwargs=None):
    x = np.asarray(x, dtype=np.float32)
    c = np.asarray(c, dtype=np.float32)
    bias = np.asarray(bias, dtype=np.float32)

    wv, wuu, wuv, betas = _build_weights(c, bias)
    ones = np.ones((65, 128), dtype=np.float32)

    xb = x.reshape(NTOK, IN_BLOCKS, B)
    u = xb[:, :, :H] + xb[:, :, H:]                      # (NTOK, I, H)
    v_all = (xb[:, :, :H] - xb[:, :, H:]).reshape(NTOK, HF)
    uu_all = (u[:, :, :Q] + u[:, :, Q:]).reshape(NTOK, QF)
    uv_all = (u[:, :, :Q] - u[:, :, Q:]).reshape(NTOK, QF)

    in_maps = []
    for cid in range(N_CORES):
        sl = slice(cid * TOK, (cid + 1) * TOK)
        in_maps.append(
            {
                "vT": _round_fp32r(v_all[sl].T),         # (HF, TOK)
                "uuT": _round_fp32r(uu_all[sl].T),       # (QF, TOK)
                "uvT": _round_fp32r(uv_all[sl].T),
                "wV": wv,
                "wUU": wuu,
                "wUV": wuv,
                "betaAll": betas,
                "ones": ones,
            }
        )

    nc = _get_nc()
    kw = dict(_spmd_kwargs or {})
    one_core = kw.pop("_one_core", False)
    if one_core:
        res = run_bass_kernel_spmd(nc, in_maps[:1], core_ids=[0], **kw)
        return None, res

    res = run_bass_kernel_spmd(
        nc, in_maps, core_ids=list(range(N_CORES)), **kw
    )

    def reassemble(a):
        # (NTV, MT, 2, 128, NW) -> (TOK, OUT_F)
        a = a.reshape(NTV, MT, 2, 128, JB, H)
        return a.transpose(1, 3, 0, 4, 2, 5).reshape(TOK, OUT_F)

    y = np.concatenate([reassemble(r["y"]) for r in res.results], axis=0)
    out = y.reshape(BATCH, SEQ, OUT_F)
    if _spmd_kwargs:
        return out, res
    return out
